# revision 15
# baseline (speedup 1.0000x reference)
"""Trainium2 Bass kernel for nn_LocalRNN (local GRU, chunked scan).

Problem: B=32, S=2048, I=H=256, ksize=16. Each ksize-chunk runs a GRU from
h0=0, so the 32*128=4096 chunks are independent length-16 GRU chains.

Sharding: data-parallel over chunks — core c gets batch rows [4c:4c+4],
i.e. 512 chains. Weights replicated.

Per-core layout ("transposed"): gate/hidden dim on partitions, chain (seq)
index on the free dim, all NS=512 chains in one matmul (N=512 = one PSUM
bank of fp32). Per step t, for each gate-half m (2 halves of 128):

  psum[gate_m, seqs] = W_ih_m @ x_t^T (+ W_hh_m @ h_{t-1}^T)   (PE, fp16)
  r = sigmoid(psum_r + b_r)                    (ScalarE bias port)
  z = sigmoid(psum_z + b_z)
  n = tanh((psum_in + b_in) + r*(psum_hn + b_hn))  (DVE stt ops + ScalarE)
  h = n + z*(h_prev - n)                       (DVE d/e/h chain)

The 8 PSUM banks hold r0,r1,z0,z1,in0,in1,hn0,hn1 single-buffered; step
t+1's x-side matmuls are emitted right after step t's h-side matmuls so the
PE pipelines across the elementwise chain (x-side needs no h). h-side
matmuls are ordered so k0-consumers lead (h half 0 lands ~1us before half 1).

Matmul operands and elementwise SBUF tensors are fp16 (values are O(1));
PSUM accumulation is fp32. Weight DMAs ride the Sync queue while x tiles
ride the GpSimd queue so the startup transfers overlap. Host pre-transposes
x / weights into DMA-friendly contiguous blocks and inverts the output
layout at the end.
"""

import sys

for _p in ("/opt/trn_rl_repo", "/root/.axon_site"):
    if _p not in sys.path:
        sys.path.insert(0, _p)

import ml_dtypes  # noqa: F401
import numpy as np

import concourse.bass as bass  # noqa: F401
import concourse.tile as tile
from concourse import bacc, mybir
from concourse.bass_utils import run_bass_kernel_spmd

# Problem constants (hardcoded per harness contract).
B, S, I, H = 32, 2048, 256, 256
KSIZE = 16
NCORES = 8
ROWS_PER_CORE = B // NCORES            # 4 batch rows per core
CHUNKS_PER_ROW = S // KSIZE            # 128
NS = ROWS_PER_CORE * CHUNKS_PER_ROW    # 512 chains per core
KT = 2                                 # contraction tiles (I/128 = H/128 = 2)

F32 = mybir.dt.float32
F16 = mybir.dt.float16
AF = mybir.ActivationFunctionType
OP = mybir.AluOpType

MM_DT = F16
NP_MM_DT = np.float16

# Gate column bases in the 3H weight layout.
GCOL = {"r": 0, "z": 256, "n": 512}


def build_nc():
    nc = bacc.Bacc("TRN2", target_bir_lowering=False, debug=False)

    # Inputs (host pre-transposed, contiguous per-DMA blocks).
    # xt[t, p, k, s] = x_shard[seq=s, t, i=k*128+p]
    xt_d = nc.dram_tensor("xt", [KSIZE, 128, KT, NS], MM_DT, kind="ExternalInput")
    # wih split so the first (x-side r/z) matmuls can start sooner.
    wih_rz_d = nc.dram_tensor("wih_rz", [128, KT, 512], MM_DT, kind="ExternalInput")
    wih_n_d = nc.dram_tensor("wih_n", [128, KT, 256], MM_DT, kind="ExternalInput")
    whh_d = nc.dram_tensor("whh_t", [128, KT, 3 * H], MM_DT, kind="ExternalInput")
    # brz[p, mi] = (b_ih+b_hh)[mi*128+p] for mi in r0,r1,z0,z1
    brz_d = nc.dram_tensor("brz", [128, 4], F32, kind="ExternalInput")
    # bhn[p, m] = b_hh[2H + m*128 + p]; bin[p, m] = b_ih[2H + m*128 + p]
    bhn_d = nc.dram_tensor("bhn", [128, 2], F32, kind="ExternalInput")
    bin_d = nc.dram_tensor("bin", [128, 2], F32, kind="ExternalInput")
    # out[t, m, p, s] = h_t[seq=s, hdim=m*128+p]
    out_d = nc.dram_tensor("out", [KSIZE, 2, 128, NS], MM_DT, kind="ExternalOutput")

    with tile.TileContext(nc) as tc:
        with (
            tc.tile_pool(name="consts", bufs=1) as consts,
            tc.tile_pool(name="xp", bufs=KSIZE) as xp,
            tc.tile_pool(name="ps", bufs=1, space="PSUM") as ps,
            tc.tile_pool(name="work", bufs=2) as work,
            tc.tile_pool(name="hp", bufs=3) as hp,
        ):
            # --- Input DMAs. Weights/biases dispatch from the Scalar queue,
            # x tiles from Sync, so the startup transfers overlap.
            wih = consts.tile([128, KT, 3 * H], MM_DT)
            nc.scalar.dma_start(wih[:, :, 0:512], wih_rz_d.ap())
            xs = []
            for t in range(KSIZE):
                xs.append(
                    xp.tile([128, KT, NS], MM_DT, tag="x", name=f"xs{t}")
                )
            nc.sync.dma_start(xs[0][:], xt_d.ap()[0])
            nc.scalar.dma_start(wih[:, :, 512:768], wih_n_d.ap())
            nc.sync.dma_start(xs[1][:], xt_d.ap()[1])
            brz = consts.tile([128, 4], F32)
            nc.scalar.dma_start(brz[:], brz_d.ap())
            bhn = consts.tile([128, 2], F32)
            nc.scalar.dma_start(bhn[:], bhn_d.ap())
            bin_ = consts.tile([128, 2], F32)
            nc.scalar.dma_start(bin_[:], bin_d.ap())
            whh = consts.tile([128, KT, 3 * H], MM_DT)
            nc.scalar.dma_start(whh[:], whh_d.ap())
            for t in range(2, KSIZE):
                nc.sync.dma_start(xs[t][:], xt_d.ap()[t])

            def new_banks():
                return {
                    q: [
                        ps.tile(
                            [128, NS], F32, tag=f"{q}{m}", name=f"bank_{q}{m}"
                        )
                        for m in range(2)
                    ]
                    for q in ("r", "z", "in", "hn")
                }

            def emit_x_mms(t, banks):
                """x-side matmuls for step t (emitted during step t-1).
                r/z first (their banks free earliest), in last."""
                stop_rz = t == 0  # no h-side at t=0: x k1 closes the group
                for q, m, stop in (
                    ("r", 0, stop_rz), ("r", 1, stop_rz),
                    ("z", 0, stop_rz), ("in", 0, True),
                    ("in", 1, True), ("z", 1, stop_rz),
                ):
                    col = slice(GCOL[q if q != "in" else "n"] + m * 128,
                                GCOL[q if q != "in" else "n"] + (m + 1) * 128)
                    for k in range(KT):
                        nc.tensor.matmul(
                            banks[q][m][:], wih[:, k, col], xs[t][:, k, :],
                            start=(k == 0), stop=(stop and k == KT - 1),
                        )

            def emit_h_mms(t, banks, hprev):
                """h-side matmuls for step t. k0-consumers first (h half 0 is
                ready ~1us before half 1); r gate leads the EW chain."""
                order = [
                    ("r", 0, 0), ("r", 1, 0), ("r", 0, 1), ("r", 1, 1),
                    ("hn", 0, 0), ("hn", 1, 0), ("hn", 0, 1), ("hn", 1, 1),
                    ("z", 0, 0), ("z", 0, 1), ("z", 1, 0), ("z", 1, 1),
                ]
                for q, m, k in order:
                    col = slice(GCOL[q if q != "hn" else "n"] + m * 128,
                                GCOL[q if q != "hn" else "n"] + (m + 1) * 128)
                    nc.tensor.matmul(
                        banks[q][m][:], whh[:, k, col], hprev[:, k, :],
                        start=(q == "hn" and k == 0), stop=(k == KT - 1),
                    )

            cur = new_banks()

            # --- PE warm-up: the HAM clock gate keeps the PE at 1.2 GHz
            # until it sees ~3.4us of sustained matmul activity. Run dummy
            # matmuls on a zeroed tile while the input DMAs are in flight so
            # the real stream starts at 2.4 GHz. The garbage written to bank
            # r0 is cleared by the first real (start=True) matmul.
            warm = consts.tile([128, 128], MM_DT)
            nc.vector.memset(warm[:], 0)
            for i in range(68):
                nc.tensor.matmul(
                    cur["r"][0][:, 0:64], warm[:], warm[:, 0:64],
                    start=(i == 0), stop=(i == 67), skip_group_check=True,
                )

            emit_x_mms(0, cur)
            hprev = None
            for t in range(KSIZE):
                if t > 0:
                    emit_h_mms(t, cur, hprev[:])
                if t < KSIZE - 1:
                    nxt = new_banks()
                    emit_x_mms(t + 1, nxt)
                else:
                    nxt = None

                # --- Elementwise chain for step t.
                r_t = work.tile([128, 2, NS], MM_DT, tag="rg")
                z_t = work.tile([128, 2, NS], MM_DT, tag="zg")
                tmp = work.tile([128, 2, NS], MM_DT, tag="tmp")
                pren = work.tile([128, 2, NS], MM_DT, tag="pren")
                n_t = work.tile([128, 2, NS], MM_DT, tag="n")
                d_t = work.tile([128, 2, NS], MM_DT, tag="d")
                e_t = work.tile([128, 2, NS], MM_DT, tag="e")
                hnew = hp.tile([128, 2, NS], MM_DT, tag="h")

                # ScalarE queue: r0, r1, z0, tanh0, tanh1, z1 — z1 moved off
                # the serial span between the r sigmoids and tanh1 (it only
                # feeds e1); z0 stays to fill the pren0 wait.
                nc.scalar.activation(
                    r_t[:, 0, :], cur["r"][0][:], AF.Sigmoid, bias=brz[:, 0:1]
                )
                nc.scalar.activation(
                    r_t[:, 1, :], cur["r"][1][:], AF.Sigmoid, bias=brz[:, 1:2]
                )
                nc.scalar.activation(
                    z_t[:, 0, :], cur["z"][0][:], AF.Sigmoid, bias=brz[:, 2:3]
                )

                for m in range(2):
                    if t == 0:
                        nc.vector.tensor_scalar_mul(
                            tmp[:, m, :], r_t[:, m, :], bhn[:, m : m + 1]
                        )
                    else:
                        nc.vector.scalar_tensor_tensor(
                            tmp[:, m, :], cur["hn"][m][:], bhn[:, m : m + 1],
                            r_t[:, m, :], op0=OP.add, op1=OP.mult,
                        )
                    nc.vector.scalar_tensor_tensor(
                        pren[:, m, :], cur["in"][m][:], bin_[:, m : m + 1],
                        tmp[:, m, :], op0=OP.add, op1=OP.add,
                    )
                    nc.scalar.activation(
                        n_t[:, m, :], pren[:, m, :], AF.Tanh
                    )
                nc.scalar.activation(
                    z_t[:, 1, :], cur["z"][1][:], AF.Sigmoid, bias=brz[:, 3:4]
                )

                # DVE tail per half: d = h_prev - n; e = z*d; h = n + e.
                # (t=0: e = z*n; h = n - e.)  Final step runs quarter-sized
                # pieces so the last output DMA starts sooner.
                spl = (
                    [(i * NS // 4, (i + 1) * NS // 4) for i in range(4)]
                    if t == KSIZE - 1 else [(0, NS)]
                )
                for m in range(2):
                    for s0, s1 in spl:
                        if t == 0:
                            nc.vector.tensor_tensor(
                                e_t[:, m, s0:s1], z_t[:, m, s0:s1],
                                n_t[:, m, s0:s1], op=OP.mult,
                            )
                            nc.vector.tensor_tensor(
                                hnew[:, m, s0:s1], n_t[:, m, s0:s1],
                                e_t[:, m, s0:s1], op=OP.subtract,
                            )
                        else:
                            nc.vector.tensor_tensor(
                                d_t[:, m, s0:s1], hprev[:, m, s0:s1],
                                n_t[:, m, s0:s1], op=OP.subtract,
                            )
                            nc.vector.tensor_tensor(
                                e_t[:, m, s0:s1], z_t[:, m, s0:s1],
                                d_t[:, m, s0:s1], op=OP.mult,
                            )
                            nc.vector.tensor_tensor(
                                hnew[:, m, s0:s1], n_t[:, m, s0:s1],
                                e_t[:, m, s0:s1], op=OP.add,
                            )
                        nc.gpsimd.dma_start(
                            out_d.ap()[t, m][:, s0:s1], hnew[:, m, s0:s1]
                        )

                hprev = hnew
                cur = nxt

    nc.compile()
    return nc


_NC_CACHE = None


def _get_nc():
    global _NC_CACHE
    if _NC_CACHE is None:
        _NC_CACHE = build_nc()
    return _NC_CACHE


def _prep_shared(W_ih, W_hh, b_ih, b_hh):
    wih_t = np.ascontiguousarray(
        W_ih.T.reshape(KT, 128, 3 * H).transpose(1, 0, 2)
    ).astype(NP_MM_DT)
    whh_t = np.ascontiguousarray(
        W_hh.T.reshape(KT, 128, 3 * H).transpose(1, 0, 2)
    ).astype(NP_MM_DT)
    wih_rz = np.ascontiguousarray(wih_t[:, :, 0:512])
    wih_n = np.ascontiguousarray(wih_t[:, :, 512:768])
    bsum = (b_ih + b_hh).astype(np.float32)
    brz = np.ascontiguousarray(bsum[: 2 * H].reshape(4, 128).T)
    bhn = np.ascontiguousarray(b_hh[2 * H :].reshape(2, 128).T)
    bin_ = np.ascontiguousarray(b_ih[2 * H :].reshape(2, 128).T)
    return wih_rz, wih_n, whh_t, brz, bhn, bin_


def _prep_core_inputs(x, shared, core):
    wih_rz, wih_n, whh_t, brz, bhn, bin_ = shared
    xc = x[core * ROWS_PER_CORE : (core + 1) * ROWS_PER_CORE]  # [4, S, I]
    xc = xc.reshape(NS, KSIZE, I)
    # xt[t, p, k, s] = xc[s, t, k*128+p]
    xt = np.ascontiguousarray(
        xc.reshape(NS, KSIZE, KT, 128).transpose(1, 3, 2, 0)
    ).astype(NP_MM_DT)
    return {
        "xt": xt,
        "wih_rz": wih_rz,
        "wih_n": wih_n,
        "whh_t": whh_t,
        "brz": brz,
        "bhn": bhn,
        "bin": bin_,
    }


def kernel(x, W_ih, W_hh, b_ih, b_hh, ksize):
    x = np.asarray(x, dtype=np.float32)
    W_ih = np.asarray(W_ih, dtype=np.float32)
    W_hh = np.asarray(W_hh, dtype=np.float32)
    b_ih = np.asarray(b_ih, dtype=np.float32)
    b_hh = np.asarray(b_hh, dtype=np.float32)
    assert int(ksize) == KSIZE and x.shape == (B, S, I)

    shared = _prep_shared(W_ih, W_hh, b_ih, b_hh)
    in_maps = [_prep_core_inputs(x, shared, c) for c in range(NCORES)]
    nc = _get_nc()
    res = run_bass_kernel_spmd(nc, in_maps, core_ids=list(range(NCORES)))

    out = np.empty((B, S, H), dtype=np.float32)
    for c in range(NCORES):
        oc = np.asarray(res.results[c]["out"]).astype(np.float32)  # [t,m,p,s]
        # h[seq=s, t, hdim=m*128+p]
        hc = oc.transpose(3, 0, 1, 2).reshape(NS, KSIZE, H)
        out[c * ROWS_PER_CORE : (c + 1) * ROWS_PER_CORE] = hc.reshape(
            ROWS_PER_CORE, S, H
        )
    return out


# revision 16
# speedup vs baseline: 1.0390x; 1.0390x over previous
"""Trainium2 Bass kernel for nn_LocalRNN (local GRU, chunked scan).

Problem: B=32, S=2048, I=H=256, ksize=16. Each ksize-chunk runs a GRU from
h0=0, so the 32*128=4096 chunks are independent length-16 GRU chains.

Sharding: data-parallel over chunks — core c gets batch rows [4c:4c+4],
i.e. 512 chains. Weights replicated.

Per-core kernel layout ("transposed"): gate/hidden dim on partitions, chain
(seq) index on the free dim. Per step t and seq-group g (2 groups x 256 seqs):

  gates[3H, seqs] = W_ih @ x_t^T + W_hh @ h_{t-1}^T     (PSUM accumulation)
  r = sigmoid(psum_r + (b_ih+b_hh)_r)                    (ScalarE, bias port)
  z = sigmoid(psum_z + (b_ih+b_hh)_z)
  n = tanh((psum_in + b_ih_n) + r*(psum_hn + b_hh_n))    (fused DVE stt ops)
  h = n + z*(h_prev - n)

The x-side and h-side matmuls for r/z accumulate into the same PSUM bank so
no explicit adds are needed; n keeps separate x/h banks because r multiplies
only the h side. PSUM budget: 4 banks per group x 2 groups = all 8 banks,
ping-ponged so one group's matmuls overlap the other group's elementwise.

Extras over the plain version: dummy warm-up matmuls run during the input
DMAs so the HAM clock gate reaches 2.4 GHz before the real stream; weights
ride the ScalarE DMA queue in parallel with x tiles on Sync (all 32 x tiles
prefetched); outputs dispatch from the GpSimd queue; the final step's
h-update and output DMA are split per gate-half to shorten the tail.

Matmul operands and SBUF elementwise tensors are fp16 (values are O(1) so
fp16 range is safe); PSUM accumulation is fp32. Host pre-transposes x /
weights into DMA-friendly contiguous blocks and inverts the output layout.
"""

import sys

for _p in ("/opt/trn_rl_repo", "/root/.axon_site"):
    if _p not in sys.path:
        sys.path.insert(0, _p)

import ml_dtypes  # noqa: F401
import numpy as np

import concourse.bass as bass  # noqa: F401
import concourse.tile as tile
from concourse import bacc, mybir
from concourse.bass_utils import run_bass_kernel_spmd

# Problem constants (hardcoded per harness contract).
B, S, I, H = 32, 2048, 256, 256
KSIZE = 16
NCORES = 8
ROWS_PER_CORE = B // NCORES            # 4 batch rows per core
CHUNKS_PER_ROW = S // KSIZE            # 128
SEQS = ROWS_PER_CORE * CHUNKS_PER_ROW  # 512 chains per core
G = 2                                  # seq groups per core
NS = SEQS // G                         # 256 seqs per group
KT = 2                                 # contraction tiles (I/128 = H/128 = 2)

F32 = mybir.dt.float32
F16 = mybir.dt.float16
AF = mybir.ActivationFunctionType
OP = mybir.AluOpType

MM_DT = F16         # matmul operand + elementwise SBUF dtype
NP_MM_DT = np.float16


def build_nc():
    nc = bacc.Bacc("TRN2", target_bir_lowering=False, debug=False)

    # Inputs (host pre-transposed, contiguous per-DMA blocks).
    # xt[t, g, p, k, s] = x_shard[seq=g*NS+s, t, i=k*128+p]
    xt_d = nc.dram_tensor("xt", [KSIZE, G, 128, KT, NS], MM_DT, kind="ExternalInput")
    # wih_t[p, k, m] = W_ih[m, k*128+p]  (transposed weight, lhsT layout),
    # split so the r/z columns land first.
    wih_rz_d = nc.dram_tensor("wih_rz", [128, KT, 512], MM_DT, kind="ExternalInput")
    wih_n_d = nc.dram_tensor("wih_n", [128, KT, 256], MM_DT, kind="ExternalInput")
    whh_d = nc.dram_tensor("whh_t", [128, KT, 3 * H], MM_DT, kind="ExternalInput")
    # brz[p, mi] = (b_ih+b_hh)[mi*128+p] for mi in 0..3 (r0,r1,z0,z1)
    brz_d = nc.dram_tensor("brz", [128, 4], F32, kind="ExternalInput")
    # bhn[p, m] = b_hh[2H + m*128 + p]; bin[p, m] = b_ih[2H + m*128 + p]
    bhn_d = nc.dram_tensor("bhn", [128, 2], F32, kind="ExternalInput")
    bin_d = nc.dram_tensor("bin", [128, 2], F32, kind="ExternalInput")
    # out[t, g, p, m, s] = h_t[seq=g*NS+s, hdim=m*128+p]
    out_d = nc.dram_tensor("out", [KSIZE, G, 128, 2, NS], MM_DT, kind="ExternalOutput")

    with tile.TileContext(nc) as tc:
        with (
            tc.tile_pool(name="consts", bufs=1) as consts,
            tc.tile_pool(name="xp", bufs=KSIZE * G) as xp,
            tc.tile_pool(name="ps", bufs=2, space="PSUM") as ps,
            tc.tile_pool(name="work", bufs=4) as work,
            tc.tile_pool(name="hp", bufs=4) as hp,
        ):
            # Input DMAs: weights/biases on the ScalarE queue, x tiles on
            # Sync, so the startup transfers overlap. First-use order.
            wih = consts.tile([128, KT, 3 * H], MM_DT)
            nc.scalar.dma_start(wih[:, :, 0:512], wih_rz_d.ap())
            xs = {}
            for t in range(KSIZE):
                for g in range(G):
                    xs[(t, g)] = xp.tile(
                        [128, KT, NS], MM_DT, tag="x", name=f"xs{t}_{g}"
                    )
            nc.sync.dma_start(xs[(0, 0)][:], xt_d.ap()[0, 0])
            nc.scalar.dma_start(wih[:, :, 512:768], wih_n_d.ap())
            nc.sync.dma_start(xs[(0, 1)][:], xt_d.ap()[0, 1])
            brz = consts.tile([128, 4], F32)
            nc.scalar.dma_start(brz[:], brz_d.ap())
            bhn = consts.tile([128, 2], F32)
            nc.scalar.dma_start(bhn[:], bhn_d.ap())
            bin_ = consts.tile([128, 2], F32)
            nc.scalar.dma_start(bin_[:], bin_d.ap())
            whh = consts.tile([128, KT, 3 * H], MM_DT)
            nc.scalar.dma_start(whh[:], whh_d.ap())
            for t in range(KSIZE):
                for g in range(G):
                    if t == 0:
                        continue
                    nc.sync.dma_start(xs[(t, g)][:], xt_d.ap()[t, g])

            h_state = [None] * G
            warm_done = False
            for t in range(KSIZE):
                for g in range(G):
                    xr = xs[(t, g)][:]
                    hr = None if t == 0 else h_state[g][:]

                    # PSUM banks: [128, 2, NS] f32 = one 2KB bank each.
                    bank_r = ps.tile([128, 2, NS], F32, tag="r", name="bank_r")
                    bank_z = ps.tile([128, 2, NS], F32, tag="z", name="bank_z")
                    bank_in = ps.tile(
                        [128, 2, NS], F32, tag="in", name="bank_in"
                    )
                    bank_hn = (
                        None if t == 0
                        else ps.tile([128, 2, NS], F32, tag="hn", name="bank_hn")
                    )

                    if not warm_done:
                        # PE warm-up while the input DMAs are in flight: the
                        # HAM clock gate holds the PE at 1.2 GHz until ~3.4us
                        # of sustained matmul activity. The garbage written
                        # to bank_r is cleared by the first start=True matmul.
                        warm = consts.tile([128, 128], MM_DT)
                        nc.vector.memset(warm[:], 0)
                        for i in range(68):
                            nc.tensor.matmul(
                                bank_r[:, 0, 0:64], warm[:], warm[:, 0:64],
                                start=(i == 0), stop=(i == 67),
                                skip_group_check=True,
                            )
                        warm_done = True

                    # Matmuls. W row tiles: r halves mi=0,1; z mi=2,3; n mi=4,5.
                    def mm_accum(bank_t, mi, m, with_h):
                        col = slice(mi * 128, (mi + 1) * 128)
                        n_mm = 2 * KT if with_h else KT
                        i_mm = 0
                        for k in range(KT):
                            nc.tensor.matmul(
                                bank_t[:, m, :], wih[:, k, col], xr[:, k, :],
                                start=(i_mm == 0), stop=(i_mm == n_mm - 1),
                            )
                            i_mm += 1
                        if with_h:
                            for k in range(KT):
                                nc.tensor.matmul(
                                    bank_t[:, m, :], whh[:, k, col], hr[:, k, :],
                                    start=False, stop=(i_mm == n_mm - 1),
                                )
                                i_mm += 1

                    def mm_h_only(bank_t, mi, m):
                        col = slice(mi * 128, (mi + 1) * 128)
                        for k in range(KT):
                            nc.tensor.matmul(
                                bank_t[:, m, :], whh[:, k, col], hr[:, k, :],
                                start=(k == 0), stop=(k == KT - 1),
                            )

                    for m in range(2):
                        mm_accum(bank_r, m, m, t > 0)
                    if t > 0:
                        for m in range(2):
                            mm_h_only(bank_hn, 4 + m, m)
                    for m in range(2):
                        mm_accum(bank_z, 2 + m, m, t > 0)
                    for m in range(2):
                        mm_accum(bank_in, 4 + m, m, False)

                    # Elementwise.
                    r_t = work.tile([128, 2, NS], MM_DT, tag="rg", name="r_t")
                    z_t = work.tile([128, 2, NS], MM_DT, tag="zg", name="z_t")
                    for mi in range(2):  # r halves first: r leads the chain
                        nc.scalar.activation(
                            r_t[:, mi, :], bank_r[:, mi, :], AF.Sigmoid,
                            bias=brz[:, mi : mi + 1],
                        )
                    for mi in range(2):  # z halves after (consumed late)
                        nc.scalar.activation(
                            z_t[:, mi, :], bank_z[:, mi, :], AF.Sigmoid,
                            bias=brz[:, 2 + mi : 3 + mi],
                        )

                    tmp = work.tile([128, 2, NS], MM_DT, tag="tmp", name="tmp")
                    pren = work.tile(
                        [128, 2, NS], MM_DT, tag="pren", name="pren"
                    )
                    for m in range(2):
                        if t == 0:
                            # h=0: h-side n contribution is just b_hh_n.
                            nc.vector.tensor_scalar_mul(
                                tmp[:, m, :], r_t[:, m, :], bhn[:, m : m + 1]
                            )
                        else:
                            # tmp = (psum_hn + b_hh_n) * r
                            nc.vector.scalar_tensor_tensor(
                                tmp[:, m, :], bank_hn[:, m, :], bhn[:, m : m + 1],
                                r_t[:, m, :], op0=OP.add, op1=OP.mult,
                            )
                        # pre_n = (psum_in + b_ih_n) + tmp
                        nc.vector.scalar_tensor_tensor(
                            pren[:, m, :], bank_in[:, m, :], bin_[:, m : m + 1],
                            tmp[:, m, :], op0=OP.add, op1=OP.add,
                        )

                    n_t = work.tile([128, 2, NS], MM_DT, tag="n", name="n_t")
                    nc.scalar.activation(n_t[:], pren[:], AF.Tanh)

                    hnew = hp.tile([128, 2, NS], MM_DT, tag="h", name="hnew")
                    e = work.tile([128, 2, NS], MM_DT, tag="e", name="e_t")
                    d = work.tile([128, 2, NS], MM_DT, tag="d", name="d_t")
                    # Final step: per-half pieces so the last output DMA
                    # starts sooner (no next step left to overlap with).
                    halves = [(0, 1), (1, 2)] if t == KSIZE - 1 else [(0, 2)]
                    for m0, m1 in halves:
                        if t == 0:
                            # h1 = n - z*n
                            nc.vector.tensor_tensor(
                                e[:, m0:m1, :], z_t[:, m0:m1, :],
                                n_t[:, m0:m1, :], op=OP.mult,
                            )
                            nc.vector.tensor_tensor(
                                hnew[:, m0:m1, :], n_t[:, m0:m1, :],
                                e[:, m0:m1, :], op=OP.subtract,
                            )
                        else:
                            # h = n + z*(h_prev - n)
                            nc.vector.tensor_tensor(
                                d[:, m0:m1, :], h_state[g][:, m0:m1, :],
                                n_t[:, m0:m1, :], op=OP.subtract,
                            )
                            nc.vector.tensor_tensor(
                                e[:, m0:m1, :], z_t[:, m0:m1, :],
                                d[:, m0:m1, :], op=OP.mult,
                            )
                            nc.vector.tensor_tensor(
                                hnew[:, m0:m1, :], e[:, m0:m1, :],
                                n_t[:, m0:m1, :], op=OP.add,
                            )
                        nc.gpsimd.dma_start(
                            out_d.ap()[t, g][:, m0:m1, :], hnew[:, m0:m1, :]
                        )
                    h_state[g] = hnew

    nc.compile()
    return nc


_NC_CACHE = None


def _get_nc():
    global _NC_CACHE
    if _NC_CACHE is None:
        _NC_CACHE = build_nc()
    return _NC_CACHE


def _prep_shared(W_ih, W_hh, b_ih, b_hh):
    wih_t = np.ascontiguousarray(
        W_ih.T.reshape(KT, 128, 3 * H).transpose(1, 0, 2)
    ).astype(NP_MM_DT)
    whh_t = np.ascontiguousarray(
        W_hh.T.reshape(KT, 128, 3 * H).transpose(1, 0, 2)
    ).astype(NP_MM_DT)
    wih_rz = np.ascontiguousarray(wih_t[:, :, 0:512])
    wih_n = np.ascontiguousarray(wih_t[:, :, 512:768])
    bsum = b_ih + b_hh
    brz = np.ascontiguousarray(bsum[: 2 * H].reshape(4, 128).T)
    bhn = np.ascontiguousarray(b_hh[2 * H :].reshape(2, 128).T)
    bin_ = np.ascontiguousarray(b_ih[2 * H :].reshape(2, 128).T)
    return wih_rz, wih_n, whh_t, brz, bhn, bin_


def _prep_core_inputs(x, shared, core):
    wih_rz, wih_n, whh_t, brz, bhn, bin_ = shared
    xc = x[core * ROWS_PER_CORE : (core + 1) * ROWS_PER_CORE]  # [4, S, I]
    xc = xc.reshape(SEQS, KSIZE, I)
    # xt[t, g, p, k, s] = xc[g*NS+s, t, k*128+p]
    xt = np.ascontiguousarray(
        xc.reshape(G, NS, KSIZE, KT, 128).transpose(2, 0, 4, 3, 1)
    ).astype(NP_MM_DT)
    return {
        "xt": xt,
        "wih_rz": wih_rz,
        "wih_n": wih_n,
        "whh_t": whh_t,
        "brz": brz,
        "bhn": bhn,
        "bin": bin_,
    }


def kernel(x, W_ih, W_hh, b_ih, b_hh, ksize):
    x = np.asarray(x, dtype=np.float32)
    W_ih = np.asarray(W_ih, dtype=np.float32)
    W_hh = np.asarray(W_hh, dtype=np.float32)
    b_ih = np.asarray(b_ih, dtype=np.float32)
    b_hh = np.asarray(b_hh, dtype=np.float32)
    assert int(ksize) == KSIZE and x.shape == (B, S, I)

    shared = _prep_shared(W_ih, W_hh, b_ih, b_hh)
    in_maps = [_prep_core_inputs(x, shared, c) for c in range(NCORES)]
    nc = _get_nc()
    res = run_bass_kernel_spmd(nc, in_maps, core_ids=list(range(NCORES)))

    out = np.empty((B, S, H), dtype=np.float32)
    for c in range(NCORES):
        oc = np.asarray(res.results[c]["out"]).astype(np.float32)  # [t,g,p,m,s]
        # h[seq=g*NS+s, t, hdim=m*128+p]
        hc = oc.transpose(1, 4, 0, 3, 2).reshape(SEQS, KSIZE, H)
        out[c * ROWS_PER_CORE : (c + 1) * ROWS_PER_CORE] = hc.reshape(
            ROWS_PER_CORE, S, H
        )
    return out
